# revision 25
# baseline (speedup 1.0000x reference)
"""Causal self-attention (B=2, S=2048, D=1024, H=16) on 8 trn2 NeuronCores.

Sharding: batch x head-group. Core c handles batch c//4 and heads
[ (c%4)*4 , (c%4)*4+4 ).  QKV projections are column-sharded, the output
projection row-sharded (Megatron style); each core produces a partial
[S, D] output (bf16) which the host sums per batch.

v2 layout strategy (everything "transposed", all matmul operands bf16):
  x^T   [D, S]   supplied pre-transposed + pre-cast by the host: the kernel
        DMAs it straight into SBUF (no PE transposes, no stage drains).
  Weights arrive pre-arranged so each is a single contiguous [128, 2048]
        bf16 DMA straight into its SBUF operand layout.
  Q^T,K^T [256, S] = W^T x^T  (lhsT = W cols, rhs = x^T chunks), bf16.
  V     [S, 256] = x W  (lhsT = x^T tiles, rhs = Wv), bf16, padded with a
        ones column per head -> AV matmul also produces the softmax
        normalizer l = sum_k exp(s) as an extra output row.
  S^T   [k, q] score chunks in PSUM; exp() applied directly (scores are
        bounded for this problem so no running-max is needed); causal mask =
        skip the fully-masked leading columns in the S/AV matmuls + one
        triangular 0/1 multiply on the diagonal 128-block of the exp output.
  out'^T [65, q] = [V|1]^T A^T accumulated over k tiles in PSUM.
  O^T = out'^T[0:64] * (1/l): 1/l (bf16) is broadcast across partitions with
        a rank-1 PE matmul (ones[1,64]^T @ rec[1,512]) instead of a DRAM
        round trip.
  out   [S, D] partial (bf16) = O^T^T Wo accumulated over 2 feature chunks.
"""

import numpy as np
import ml_dtypes

import concourse.bass as bass
import concourse.mybir as mybir
import concourse.tile as tile
from concourse.bass_utils import run_bass_kernel_spmd

B, S, D = 2, 2048, 1024
HPG, DH = 4, 64            # heads per core, head dim
OC = HPG * DH              # 256 projection cols per core
VW = DH + 1                # V padded with ones column
NT = S // 128              # 16 token tiles
NM = D // 128              # 8 dmodel chunks
QC = 512                   # q chunk width
NQC = S // QC              # 4 q chunks
F32 = mybir.dt.float32
BF16 = mybir.dt.bfloat16
bf16 = ml_dtypes.bfloat16

_NC_CACHE = {}


WAIT_CAP = 1


def _split_waits_bir(bir_json, cap=WAIT_CAP):
    """This container's walrus rejects instructions carrying more than `cap`
    sync waits.  Hoist the excess into standalone same-engine EventSemaphore
    wait ops immediately before the instruction (sequencers execute in
    order, so semantics are identical)."""
    import json as _json

    d = _json.loads(bir_json)
    n_split = 0
    for f in d.get("functions", []):
        for bb in f.get("blocks", []):
            insts = bb.get("instructions", [])
            out = []
            for inst in insts:
                si = inst.get("sync_info")
                ow = (si or {}).get("on_wait") or []
                sem_w = [w for w in ow if w.get("sync_type") == "semaphore"]
                other_w = [w for w in ow if w.get("sync_type") != "semaphore"]
                budget = max(cap - len(other_w), 0)
                if len(sem_w) > budget:
                    keep = sem_w[:budget] if budget else []
                    extra = sem_w[budget:]
                    step = max(cap, 1)
                    for i in range(0, len(extra), step):
                        n_split += 1
                        out.append({
                            "debug": inst.get("debug"),
                            "engine": inst["engine"],
                            "ins": [],
                            "name": f"{inst['name']}_sw{i}",
                            "opcode": "EventSemaphore",
                            "outs": [],
                            "sync_info": {"on_update": [],
                                          "on_wait": extra[i:i + step]},
                        })
                    si["on_wait"] = other_w + keep
                out.append(inst)
            bb["instructions"] = out
    return _json.dumps(d).encode(), n_split


def _patch_compile_hook():
    import concourse.bass_utils as bu
    import concourse.bass2jax as b2j

    orig = bu.compile_bir_kernel
    if getattr(orig, "_split_waits_wrapped", False):
        return

    def wrapped(bir_json, tmpdir, neff_name="file.neff"):
        if isinstance(bir_json, str):
            bir_json = bir_json.encode()
        bir_json, _ = _split_waits_bir(bir_json)
        return orig(bir_json, tmpdir, neff_name)

    wrapped._split_waits_wrapped = True
    bu.compile_bir_kernel = wrapped
    if getattr(b2j, "compile_bir_kernel", None) is orig:
        b2j.compile_bir_kernel = wrapped


def _patch_tile_drain():
    """This container's walrus rejects >2 sync waits on one SP CTRL op; the
    stock Tile exit drain carries one wait per active proc.  Emit separate
    single-wait instructions instead."""
    from concourse.vector_clock import ScopedClock  # noqa: F401

    def _drain_split(self, tick_clock, wait_clock):
        nc = self.nc
        sems = wait_clock.sems.allocated()
        for proc, t in enumerate(list(tick_clock.global_clock)):
            if t <= 0:
                continue
            sem = sems.get(proc)
            if sem is None:
                continue
            nc.sync.wait_ge(sem, t * (16 if sem.name.startswith("DMA") else 1))
        nc.sync.drain()
        nc.all_engine_barrier()
        popped = nc._tile_sem_poison_stack.pop()
        assert popped is self._sem_poison
        nc.clear_and_free_semaphores(list(self.sems.allocated().values()))
        nc.all_engine_barrier()

    tile.TileContext._drain_and_barrier = _drain_split


def _bc(ap, n):
    """Broadcast a [1, ...] DRAM AP across n partitions (step-0 partition)."""
    return bass.AP(tensor=ap.tensor, offset=ap.offset, ap=[[0, n]] + list(ap.ap)[1:])


def build_nc(tri_engine='pool', ob_drain='dve', lead=0.15):
    nc = bass.Bass()
    xt_d = nc.dram_tensor("xt", [D, S], BF16, kind="ExternalInput")
    wq_d = nc.dram_tensor("wq", [128, NM * OC], BF16, kind="ExternalInput")
    wk_d = nc.dram_tensor("wk", [128, NM * OC], BF16, kind="ExternalInput")
    wv_d = nc.dram_tensor("wv", [128, NM * OC], BF16, kind="ExternalInput")
    wo_d = nc.dram_tensor("wo", [128, 2 * D], BF16, kind="ExternalInput")
    bq_d = nc.dram_tensor("bq", [OC], F32, kind="ExternalInput")
    bk_d = nc.dram_tensor("bk", [OC], F32, kind="ExternalInput")
    bv_d = nc.dram_tensor("bv", [OC], F32, kind="ExternalInput")
    out_d = nc.dram_tensor("out", [S, D], BF16, kind="ExternalOutput")

    bqr = bq_d.rearrange("(p one) -> p one", one=1)
    bkr = bk_d.rearrange("(p one) -> p one", one=1)
    bvr = bv_d.rearrange("(one c) -> one c", one=1)

    with tile.TileContext(nc) as tc:
        with (
            tc.tile_pool(name="singles", bufs=1) as sing,
            tc.tile_pool(name="persist", bufs=1) as per,
            tc.tile_pool(name="apool", bufs=8) as apool,
            tc.tile_pool(name="rpool", bufs=2) as rpool,
            tc.tile_pool(name="opool", bufs=3) as opool,
            tc.tile_pool(name="pp", bufs=2, space="PSUM") as pp,
        ):
            wq_sb = sing.tile([128, NM, OC], BF16, tag="wq")
            wk_sb = sing.tile([128, NM, OC], BF16, tag="wk")
            wv_sb = sing.tile([128, NM, OC], BF16, tag="wv")
            wo_sb = sing.tile([128, 2, D], BF16, tag="wo")
            xt = per.tile([128, NM, S], BF16, tag="xt")

            # Weight DMAs first (small, needed by the first projections), then
            # x^T in S-halves so group-0/1 projections start after ~3 MB of
            # input instead of the full 5.5 MB; projections consume chunk kc
            # as it lands.  Input loads alternate between the two HWDGE rings
            # (qSPDynamicHW via nc.sync, qActDynamicHW via nc.scalar) so
            # descriptor generation for the startup burst runs in parallel.
            rings = [nc.sync, nc.scalar]

            def in_dma(i, out, in_):
                rings[i % 2].dma_start(out=out, in_=in_)

            in_dma(0, wq_sb.rearrange("p c n -> p (c n)"), wq_d[:, :])
            in_dma(1, xt[:, 0, 0:S // 2], xt_d[0:128, 0:S // 2])
            in_dma(0, wk_sb.rearrange("p c n -> p (c n)"), wk_d[:, :])
            in_dma(1, wv_sb.rearrange("p c n -> p (c n)"), wv_d[:, :])
            for kc in range(1, NM):
                in_dma(kc, xt[:, kc, 0:S // 2],
                       xt_d[kc * 128:(kc + 1) * 128, 0:S // 2])

            bq_sb = sing.tile([128, 2], F32, tag="bq")
            bk_sb = sing.tile([128, 2], F32, tag="bk")
            for o in range(2):
                nc.sync.dma_start(out=bq_sb[:, o:o + 1], in_=bqr[o * 128:(o + 1) * 128, :])
                nc.scalar.dma_start(out=bk_sb[:, o:o + 1], in_=bkr[o * 128:(o + 1) * 128, :])
            bv_sb = sing.tile([128, OC], F32, tag="bv")
            nc.sync.dma_start(out=bv_sb, in_=_bc(bvr[0:1, :], 128))
            bv4 = bv_sb.rearrange("p (h c) -> p h c", h=HPG)

            for kc in range(NM):
                in_dma(kc, xt[:, kc, S // 2:S],
                       xt_d[kc * 128:(kc + 1) * 128, S // 2:S])
            nc.sync.dma_start(out=wo_sb.rearrange("p c n -> p (c n)"),
                              in_=wo_d[:, :])

            ones1 = sing.tile([1, DH], BF16, tag="ones1")
            nc.vector.memset(ones1, 1.0)
            if tri_engine == 'dve':
                tri = sing.tile([128, 128], BF16, tag="tri")
                nc.vector.memset(tri, 1.0)
                nc.gpsimd.affine_select(
                    out=tri, in_=tri, compare_op=mybir.AluOpType.is_ge,
                    fill=0.0, base=0, channel_multiplier=-1, pattern=[[1, 128]])

            qt = [per.tile([128, S], BF16, tag=f"qt{o}", name=f"qt{o}") for o in range(2)]
            kt_ = [per.tile([128, S], BF16, tag=f"kt{o}", name=f"kt{o}") for o in range(2)]
            ot_ = [per.tile([128, S], BF16, tag=f"ot{o}", name=f"ot{o}") for o in range(2)]
            vsb = [per.tile([128, HPG, VW], BF16, tag=f"v{t}", name=f"v{t}") for t in range(NT)]
            for t in range(NT):
                nc.gpsimd.memset(vsb[t][:, :, DH:VW], 1.0)

            # Software pipeline over 4 token groups: group g's attention is
            # interleaved with group g+1's projections and the tail groups'
            # output projections so the (in-order) PE stream always has
            # non-attention work to run while ACT evaluates exp().
            def interleave(*lists, lead=0.0):
                # lead > 0 front-loads the FIRST list (attention items) so
                # ACT gets score tiles to exp() right at the round start.
                import heapq
                h, out = [], []
                for li, L in enumerate(lists):
                    if L:
                        start = -lead if li == 0 else 0.0
                        heapq.heappush(h, (start, li, 0))
                while h:
                    pos, li, idx = heapq.heappop(h)
                    out.append(lists[li][idx])
                    if idx + 1 < len(lists[li]):
                        heapq.heappush(h, (pos + 1.0 / len(lists[li]), li, idx + 1))
                return out

            def windowed(*windows):
                # windows: (items, start, end) — item i of a list sits at
                # position start + (i + 1) * (end - start) / len; merged by
                # position (stable for equal positions by list order).
                entries = []
                for li, (L, s, e) in enumerate(windows):
                    n = len(L)
                    for i, it in enumerate(L):
                        entries.append((s + (i + 1) * (e - s) / n, li, i, it))
                entries.sort(key=lambda t: (t[0], t[1], t[2]))
                return [t[3] for t in entries]

            def ab_items(g, v_chunk_out=None):
                items = []
                qk_ps = {}

                def qk_chunk(wsb, bsb, dst, o, half):
                    def f():
                        if half == 0:
                            qk_ps[(id(wsb), o)] = pp.tile(
                                [128, QC], F32, tag="gp", name=f"qk{g}_{o}")
                        ps = qk_ps[(id(wsb), o)]
                        for kc in range(4 * half, 4 * half + 4):
                            nc.tensor.matmul(
                                ps,
                                lhsT=wsb[:, kc, o * 128:(o + 1) * 128],
                                rhs=xt[:, kc, g * QC:(g + 1) * QC],
                                start=(kc == 0), stop=(kc == NM - 1))
                        if half == 1:
                            nc.vector.tensor_scalar_add(
                                out=dst[o][:, g * QC:(g + 1) * QC],
                                in0=ps, scalar1=bsb[:, o:o + 1])
                    return f
                # o-major: Q/K for o=0 complete first so heads 0/1 scores
                # (and their exp stream) can start before o=1 / V work runs
                for o in range(2):
                    for wsb, bsb, dst in ((wq_sb, bq_sb, qt), (wk_sb, bk_sb, kt_)):
                        for half in range(2):
                            items.append(qk_chunk(wsb, bsb, dst, o, half))

                v_ps = {}

                def v_chunk(tt, half=None):
                    def f():
                        if half in (0, None):
                            v_ps[tt] = pp.tile([128, OC], F32, tag="gp",
                                               name=f"pv{tt}")
                        pv = v_ps[tt]
                        kcs = (range(NM) if half is None
                               else range(4 * half, 4 * half + 4))
                        for kc in kcs:
                            nc.tensor.matmul(
                                pv,
                                lhsT=xt[:, kc, tt * 128:(tt + 1) * 128],
                                rhs=wv_sb[:, kc, :],
                                start=(kc == 0), stop=(kc == NM - 1))
                        if half in (1, None):
                            v4 = vsb[tt]
                            nc.vector.tensor_add(
                                out=v4[:, :, 0:DH],
                                in0=pv.rearrange("p (h c) -> p h c", h=HPG),
                                in1=bv4)
                    return f
                if v_chunk_out is not None:
                    v_chunk_out.append(v_chunk)
                    for tt in range(4 * g, 4 * g + 4):
                        items.append(None)  # V items emitted by caller
                    items = [i for i in items if i is not None]
                else:
                    for tt in range(4 * g, 4 * g + 4):
                        items.append(v_chunk(tt))
                return items

            def c_items(qc):
                items = []
                nkt = 4 * qc + 4
                pavs = {}

                def pair_step(h, ktp):
                    o, r = h // 2, (h % 2) * 64
                    def f():
                        qt_h = qt[o][r:r + 64, :]
                        kt_h = kt_[o][r:r + 64, :]
                        if ktp == 0:
                            pavs[h] = pp.tile([VW, QC], F32, tag="pav",
                                              bufs=2, name=f"pav{qc}_{h}")
                        pav = pavs[h]
                        kts = [k for k in (ktp, ktp + 1) if k < nkt]
                        w = 512 * len(kts)
                        ps = pp.tile([128, 1024], F32, tag="ps",
                                     name=f"ps{qc}_{h}_{ktp}")
                        offs = [max(k * 128 - qc * QC, 0) for k in kts]
                        for i, k in enumerate(kts):
                            nc.tensor.matmul(
                                ps[:, i * 512 + offs[i]:(i + 1) * 512],
                                lhsT=kt_h[:, k * 128:(k + 1) * 128],
                                rhs=qt_h[:, qc * QC + offs[i]:(qc + 1) * QC],
                                start=True, stop=True)
                        at = apool.tile([128, 1024], BF16, tag="at",
                                        name=f"at{qc}_{h}_{ktp}")
                        nc.scalar.activation(
                            out=at[:, :w], in_=ps[:, :w],
                            func=mybir.ActivationFunctionType.Exp,
                            scale=1.0 / 8.0)
                        for i, k in enumerate(kts):
                            off = offs[i]
                            if k * 128 - qc * QC >= 0:
                                # causal mask on the diagonal 128-block:
                                # keep q >= k (f >= p), zero the rest
                                blk = at[:, i * 512 + off:i * 512 + off + 128]
                                if tri_engine == 'pool':
                                    nc.gpsimd.affine_select(
                                        out=blk, in_=blk,
                                        compare_op=mybir.AluOpType.is_ge,
                                        fill=0.0, base=0, channel_multiplier=-1,
                                        pattern=[[1, 128]])
                                else:
                                    nc.vector.tensor_mul(out=blk, in0=blk, in1=tri)
                            nc.tensor.matmul(
                                pav[:, off:QC],
                                lhsT=vsb[k][:, h, :],
                                rhs=at[:, i * 512 + off:(i + 1) * 512],
                                start=(k == 0), stop=(k == nkt - 1))
                    return f

                def norm_step(h):
                    o, r = h // 2, (h % 2) * 64
                    def f():
                        pav = pavs[h]
                        rec = rpool.tile([1, QC], BF16, tag="rec",
                                         name=f"rec{qc}_{h}")
                        with nc.allow_low_precision(
                                reason="softmax normalizer bf16; matches "
                                       "kernel-wide bf16 error budget"):
                            nc.vector.reciprocal(out=rec, in_=pav[DH:VW, :])
                        recb = pp.tile([DH, QC], F32, tag="gp",
                                       name=f"recb{qc}_{h}")
                        nc.tensor.matmul(recb, lhsT=ones1, rhs=rec,
                                         start=True, stop=True)
                        rb = rpool.tile([DH, QC], BF16, tag="rb",
                                        name=f"rb{qc}_{h}")
                        nc.vector.tensor_copy(out=rb, in_=recb)
                        nc.vector.tensor_mul(
                            out=ot_[o][r:r + 64, qc * QC:(qc + 1) * QC],
                            in0=pav[0:DH, :], in1=rb)
                    return f

                for h in range(HPG):
                    for ktp in range(0, nkt, 2):
                        items.append(pair_step(h, ktp))
                    items.append(norm_step(h))
                return items

            def d_items(g, drain_eng='dve'):
                items = []

                def out_tile(tt):
                    def f():
                        ob = opool.tile([128, D], BF16, tag="ob", name=f"ob{tt}")
                        for nb in range(2):
                            po = pp.tile([128, 512], F32, tag="gp",
                                         name=f"po{tt}_{nb}")
                            for cb in range(2):
                                nc.tensor.matmul(
                                    po,
                                    lhsT=ot_[cb][:, tt * 128:(tt + 1) * 128],
                                    rhs=wo_sb[:, cb, nb * 512:(nb + 1) * 512],
                                    start=(cb == 0), stop=(cb == 1))
                            if drain_eng == 'act' or (
                                    drain_eng == 'alt' and (tt + nb) % 2 == 1):
                                nc.scalar.copy(
                                    out=ob[:, nb * 512:(nb + 1) * 512], in_=po)
                            else:
                                nc.vector.tensor_copy(
                                    out=ob[:, nb * 512:(nb + 1) * 512], in_=po)
                        nc.sync.dma_start(
                            out=out_d[tt * 128:(tt + 1) * 128, :], in_=ob)
                    return f
                for tt in range(4 * g, 4 * g + 4):
                    items.append(out_tile(tt))
                return items

            # round 0: group 0 projections alone
            for f in ab_items(0):
                f()
            # rounds 1..3: attention(r-1) interleaved with projections(r)
            for r in range(1, NQC):
                for f in interleave(c_items(r - 1), ab_items(r), lead=lead):
                    f()
            # final attention group interleaved with the first 3 groups'
            # output projections (their PE work fills exp() stalls);
            # the trailing group's drains go to ACT, idle once exps finish
            dfill = (d_items(0, drain_eng=ob_drain)
                     + d_items(1, drain_eng=ob_drain)
                     + d_items(2, drain_eng=ob_drain))
            for f in interleave(c_items(NQC - 1), dfill, lead=lead):
                f()
            for f in d_items(NQC - 1, drain_eng='act'):
                f()
    return nc


BUILD_OPTS = dict(tri_engine='dve', ob_drain='dve', lead=0.1)


def _get_nc():
    key = str(sorted(BUILD_OPTS.items()))
    if key not in _NC_CACHE:
        _patch_tile_drain()
        _patch_compile_hook()
        _NC_CACHE[key] = build_nc(**BUILD_OPTS)
    return _NC_CACHE[key]


def make_in_maps(inputs):
    x = np.asarray(inputs["x"], dtype=np.float32)
    Wq = np.asarray(inputs["Wq"], dtype=np.float32)
    Wk = np.asarray(inputs["Wk"], dtype=np.float32)
    Wv = np.asarray(inputs["Wv"], dtype=np.float32)
    Wo = np.asarray(inputs["Wo"], dtype=np.float32)
    bq = np.asarray(inputs["bq"], dtype=np.float32)
    bk = np.asarray(inputs["bk"], dtype=np.float32)
    bv = np.asarray(inputs["bv"], dtype=np.float32)

    # x^T per batch, bf16, contiguous: [D, S]
    xT = [np.ascontiguousarray(x[b].T.astype(bf16)) for b in range(B)]

    def w_cols(W, cols):
        # [D, 256] -> SBUF layout [128, 8, 256] flattened to [128, 2048]
        w = W[:, cols].astype(bf16)
        return np.ascontiguousarray(
            w.reshape(NM, 128, OC).transpose(1, 0, 2).reshape(128, NM * OC))

    def w_rows(W, rows):
        # [256, D] -> SBUF layout [128, 2, 1024] flattened to [128, 2048]
        w = W[rows, :].astype(bf16)
        return np.ascontiguousarray(
            w.reshape(2, 128, D).transpose(1, 0, 2).reshape(128, 2 * D))

    # per-head-group tensors are shared by the two batch cores (c and c+4)
    gmaps = []
    for g in range(4):
        cols = slice(g * OC, (g + 1) * OC)
        gmaps.append({
            "wq": w_cols(Wq, cols),
            "wk": w_cols(Wk, cols),
            "wv": w_cols(Wv, cols),
            "wo": w_rows(Wo, cols),
            "bq": np.ascontiguousarray(bq[cols]),
            "bk": np.ascontiguousarray(bk[cols]),
            "bv": np.ascontiguousarray(bv[cols]),
        })
    return [{"xt": xT[c // 4], **gmaps[c % 4]} for c in range(8)]


def combine(results, inputs):
    bo = np.asarray(inputs["bo"], dtype=np.float32)
    out = np.zeros((B, S, D), dtype=np.float32)
    for c in range(8):
        out[c // 4] += results[c]["out"].astype(np.float32)
    out += bo[None, None, :]
    return out


def kernel(**inputs) -> np.ndarray:
    nc = _get_nc()
    in_maps = make_in_maps(inputs)
    res = run_bass_kernel_spmd(nc, in_maps, core_ids=list(range(8)))
    return combine(res.results, inputs)


if __name__ == "__main__":
    import jax
    print(jax.devices())


# revision 27
# speedup vs baseline: 1.0026x; 1.0026x over previous
"""Causal self-attention (B=2, S=2048, D=1024, H=16) on 8 trn2 NeuronCores.

Sharding: batch x head-group. Core c handles batch c//4 and heads
[ (c%4)*4 , (c%4)*4+4 ).  QKV projections are column-sharded, the output
projection row-sharded (Megatron style); each core produces a partial
[S, D] output (bf16) which the host sums per batch.

v2 layout strategy (everything "transposed", all matmul operands bf16):
  x^T   [D, S]   supplied pre-transposed + pre-cast by the host: the kernel
        DMAs it straight into SBUF (no PE transposes, no stage drains).
  Weights arrive pre-arranged so each is a single contiguous [128, 2048]
        bf16 DMA straight into its SBUF operand layout.
  Q^T,K^T [256, S] = W^T x^T  (lhsT = W cols, rhs = x^T chunks), bf16.
  V     [S, 256] = x W  (lhsT = x^T tiles, rhs = Wv), bf16, padded with a
        ones column per head -> AV matmul also produces the softmax
        normalizer l = sum_k exp(s) as an extra output row.
  S^T   [k, q] score chunks in PSUM; exp() applied directly (scores are
        bounded for this problem so no running-max is needed); causal mask =
        skip the fully-masked leading columns in the S/AV matmuls + one
        triangular 0/1 multiply on the diagonal 128-block of the exp output.
  out'^T [65, q] = [V|1]^T A^T accumulated over k tiles in PSUM.
  O^T = out'^T[0:64] * (1/l): 1/l (bf16) is broadcast across partitions with
        a rank-1 PE matmul (ones[1,64]^T @ rec[1,512]) instead of a DRAM
        round trip.
  out   [S, D] partial (bf16) = O^T^T Wo accumulated over 2 feature chunks.
"""

import numpy as np
import ml_dtypes

import concourse.bass as bass
import concourse.mybir as mybir
import concourse.tile as tile
from concourse.bass_utils import run_bass_kernel_spmd

B, S, D = 2, 2048, 1024
HPG, DH = 4, 64            # heads per core, head dim
OC = HPG * DH              # 256 projection cols per core
VW = DH + 1                # V padded with ones column
NT = S // 128              # 16 token tiles
NM = D // 128              # 8 dmodel chunks
QC = 512                   # q chunk width
NQC = S // QC              # 4 q chunks
F32 = mybir.dt.float32
BF16 = mybir.dt.bfloat16
bf16 = ml_dtypes.bfloat16

_NC_CACHE = {}


WAIT_CAP = 1


def _split_waits_bir(bir_json, cap=WAIT_CAP):
    """This container's walrus rejects instructions carrying more than `cap`
    sync waits.  Hoist the excess into standalone same-engine EventSemaphore
    wait ops immediately before the instruction (sequencers execute in
    order, so semantics are identical)."""
    import json as _json

    d = _json.loads(bir_json)
    n_split = 0
    for f in d.get("functions", []):
        for bb in f.get("blocks", []):
            insts = bb.get("instructions", [])
            out = []
            for inst in insts:
                si = inst.get("sync_info")
                ow = (si or {}).get("on_wait") or []
                sem_w = [w for w in ow if w.get("sync_type") == "semaphore"]
                other_w = [w for w in ow if w.get("sync_type") != "semaphore"]
                budget = max(cap - len(other_w), 0)
                if len(sem_w) > budget:
                    keep = sem_w[:budget] if budget else []
                    extra = sem_w[budget:]
                    step = max(cap, 1)
                    for i in range(0, len(extra), step):
                        n_split += 1
                        out.append({
                            "debug": inst.get("debug"),
                            "engine": inst["engine"],
                            "ins": [],
                            "name": f"{inst['name']}_sw{i}",
                            "opcode": "EventSemaphore",
                            "outs": [],
                            "sync_info": {"on_update": [],
                                          "on_wait": extra[i:i + step]},
                        })
                    si["on_wait"] = other_w + keep
                out.append(inst)
            bb["instructions"] = out
    return _json.dumps(d).encode(), n_split


def _patch_compile_hook():
    import concourse.bass_utils as bu
    import concourse.bass2jax as b2j

    orig = bu.compile_bir_kernel
    if getattr(orig, "_split_waits_wrapped", False):
        return

    def wrapped(bir_json, tmpdir, neff_name="file.neff"):
        if isinstance(bir_json, str):
            bir_json = bir_json.encode()
        bir_json, _ = _split_waits_bir(bir_json)
        return orig(bir_json, tmpdir, neff_name)

    wrapped._split_waits_wrapped = True
    bu.compile_bir_kernel = wrapped
    if getattr(b2j, "compile_bir_kernel", None) is orig:
        b2j.compile_bir_kernel = wrapped


def _patch_tile_drain():
    """This container's walrus rejects >2 sync waits on one SP CTRL op; the
    stock Tile exit drain carries one wait per active proc.  Emit separate
    single-wait instructions instead."""
    from concourse.vector_clock import ScopedClock  # noqa: F401

    def _drain_split(self, tick_clock, wait_clock):
        nc = self.nc
        sems = wait_clock.sems.allocated()
        for proc, t in enumerate(list(tick_clock.global_clock)):
            if t <= 0:
                continue
            sem = sems.get(proc)
            if sem is None:
                continue
            nc.sync.wait_ge(sem, t * (16 if sem.name.startswith("DMA") else 1))
        nc.sync.drain()
        nc.all_engine_barrier()
        popped = nc._tile_sem_poison_stack.pop()
        assert popped is self._sem_poison
        nc.clear_and_free_semaphores(list(self.sems.allocated().values()))
        nc.all_engine_barrier()

    tile.TileContext._drain_and_barrier = _drain_split


def _bc(ap, n):
    """Broadcast a [1, ...] DRAM AP across n partitions (step-0 partition)."""
    return bass.AP(tensor=ap.tensor, offset=ap.offset, ap=[[0, n]] + list(ap.ap)[1:])


def build_nc(tri_engine='pool', ob_drain='dve', lead=0.15, r0merge=False, rbufs=2, obufs=3):
    nc = bass.Bass()
    xt_d = nc.dram_tensor("xt", [D, S], BF16, kind="ExternalInput")
    wq_d = nc.dram_tensor("wq", [128, NM * OC], BF16, kind="ExternalInput")
    wk_d = nc.dram_tensor("wk", [128, NM * OC], BF16, kind="ExternalInput")
    wv_d = nc.dram_tensor("wv", [128, NM * OC], BF16, kind="ExternalInput")
    wo_d = nc.dram_tensor("wo", [128, 2 * D], BF16, kind="ExternalInput")
    bq_d = nc.dram_tensor("bq", [OC], F32, kind="ExternalInput")
    bk_d = nc.dram_tensor("bk", [OC], F32, kind="ExternalInput")
    bv_d = nc.dram_tensor("bv", [OC], F32, kind="ExternalInput")
    out_d = nc.dram_tensor("out", [S, D], BF16, kind="ExternalOutput")

    bqr = bq_d.rearrange("(p one) -> p one", one=1)
    bkr = bk_d.rearrange("(p one) -> p one", one=1)
    bvr = bv_d.rearrange("(one c) -> one c", one=1)

    with tile.TileContext(nc) as tc:
        with (
            tc.tile_pool(name="singles", bufs=1) as sing,
            tc.tile_pool(name="persist", bufs=1) as per,
            tc.tile_pool(name="apool", bufs=8) as apool,
            tc.tile_pool(name="rpool", bufs=rbufs) as rpool,
            tc.tile_pool(name="opool", bufs=obufs) as opool,
            tc.tile_pool(name="pp", bufs=2, space="PSUM") as pp,
        ):
            wq_sb = sing.tile([128, NM, OC], BF16, tag="wq")
            wk_sb = sing.tile([128, NM, OC], BF16, tag="wk")
            wv_sb = sing.tile([128, NM, OC], BF16, tag="wv")
            wo_sb = sing.tile([128, 2, D], BF16, tag="wo")
            xt = per.tile([128, NM, S], BF16, tag="xt")

            # Weight DMAs first (small, needed by the first projections), then
            # x^T in S-halves so group-0/1 projections start after ~3 MB of
            # input instead of the full 5.5 MB; projections consume chunk kc
            # as it lands.  Input loads alternate between the two HWDGE rings
            # (qSPDynamicHW via nc.sync, qActDynamicHW via nc.scalar) so
            # descriptor generation for the startup burst runs in parallel.
            rings = [nc.sync, nc.scalar]

            def in_dma(i, out, in_):
                rings[i % 2].dma_start(out=out, in_=in_)

            in_dma(0, wq_sb.rearrange("p c n -> p (c n)"), wq_d[:, :])
            in_dma(1, xt[:, 0, 0:S // 2], xt_d[0:128, 0:S // 2])
            in_dma(0, wk_sb.rearrange("p c n -> p (c n)"), wk_d[:, :])
            in_dma(1, wv_sb.rearrange("p c n -> p (c n)"), wv_d[:, :])
            for kc in range(1, NM):
                in_dma(kc, xt[:, kc, 0:S // 2],
                       xt_d[kc * 128:(kc + 1) * 128, 0:S // 2])

            bq_sb = sing.tile([128, 2], F32, tag="bq")
            bk_sb = sing.tile([128, 2], F32, tag="bk")
            for o in range(2):
                nc.sync.dma_start(out=bq_sb[:, o:o + 1], in_=bqr[o * 128:(o + 1) * 128, :])
                nc.scalar.dma_start(out=bk_sb[:, o:o + 1], in_=bkr[o * 128:(o + 1) * 128, :])
            bv_sb = sing.tile([128, OC], F32, tag="bv")
            nc.sync.dma_start(out=bv_sb, in_=_bc(bvr[0:1, :], 128))
            bv4 = bv_sb.rearrange("p (h c) -> p h c", h=HPG)

            for kc in range(NM):
                in_dma(kc, xt[:, kc, S // 2:S],
                       xt_d[kc * 128:(kc + 1) * 128, S // 2:S])
            nc.sync.dma_start(out=wo_sb.rearrange("p c n -> p (c n)"),
                              in_=wo_d[:, :])

            ones1 = sing.tile([1, DH], BF16, tag="ones1")
            nc.vector.memset(ones1, 1.0)
            if tri_engine == 'dve':
                tri = sing.tile([128, 128], BF16, tag="tri")
                nc.vector.memset(tri, 1.0)
                nc.gpsimd.affine_select(
                    out=tri, in_=tri, compare_op=mybir.AluOpType.is_ge,
                    fill=0.0, base=0, channel_multiplier=-1, pattern=[[1, 128]])

            qt = [per.tile([128, S], BF16, tag=f"qt{o}", name=f"qt{o}") for o in range(2)]
            kt_ = [per.tile([128, S], BF16, tag=f"kt{o}", name=f"kt{o}") for o in range(2)]
            ot_ = [per.tile([128, S], BF16, tag=f"ot{o}", name=f"ot{o}") for o in range(2)]
            vsb = [per.tile([128, HPG, VW], BF16, tag=f"v{t}", name=f"v{t}") for t in range(NT)]
            for t in range(NT):
                nc.gpsimd.memset(vsb[t][:, :, DH:VW], 1.0)

            # Software pipeline over 4 token groups: group g's attention is
            # interleaved with group g+1's projections and the tail groups'
            # output projections so the (in-order) PE stream always has
            # non-attention work to run while ACT evaluates exp().
            def interleave(*lists, lead=0.0):
                # lead > 0 front-loads the FIRST list (attention items) so
                # ACT gets score tiles to exp() right at the round start.
                import heapq
                h, out = [], []
                for li, L in enumerate(lists):
                    if L:
                        start = -lead if li == 0 else 0.0
                        heapq.heappush(h, (start, li, 0))
                while h:
                    pos, li, idx = heapq.heappop(h)
                    out.append(lists[li][idx])
                    if idx + 1 < len(lists[li]):
                        heapq.heappush(h, (pos + 1.0 / len(lists[li]), li, idx + 1))
                return out

            def windowed(*windows):
                # windows: (items, start, end) — item i of a list sits at
                # position start + (i + 1) * (end - start) / len; merged by
                # position (stable for equal positions by list order).
                entries = []
                for li, (L, s, e) in enumerate(windows):
                    n = len(L)
                    for i, it in enumerate(L):
                        entries.append((s + (i + 1) * (e - s) / n, li, i, it))
                entries.sort(key=lambda t: (t[0], t[1], t[2]))
                return [t[3] for t in entries]

            def ab_items(g, v_chunk_out=None):
                items = []
                qk_ps = {}

                def qk_chunk(wsb, bsb, dst, o, half):
                    def f():
                        if half == 0:
                            qk_ps[(id(wsb), o)] = pp.tile(
                                [128, QC], F32, tag="gp", name=f"qk{g}_{o}")
                        ps = qk_ps[(id(wsb), o)]
                        for kc in range(4 * half, 4 * half + 4):
                            nc.tensor.matmul(
                                ps,
                                lhsT=wsb[:, kc, o * 128:(o + 1) * 128],
                                rhs=xt[:, kc, g * QC:(g + 1) * QC],
                                start=(kc == 0), stop=(kc == NM - 1))
                        if half == 1:
                            nc.vector.tensor_scalar_add(
                                out=dst[o][:, g * QC:(g + 1) * QC],
                                in0=ps, scalar1=bsb[:, o:o + 1])
                    return f
                # o-major: Q/K for o=0 complete first so heads 0/1 scores
                # (and their exp stream) can start before o=1 / V work runs
                for o in range(2):
                    for wsb, bsb, dst in ((wq_sb, bq_sb, qt), (wk_sb, bk_sb, kt_)):
                        for half in range(2):
                            items.append(qk_chunk(wsb, bsb, dst, o, half))

                v_ps = {}

                def v_chunk(tt, half=None):
                    def f():
                        if half in (0, None):
                            v_ps[tt] = pp.tile([128, OC], F32, tag="gp",
                                               name=f"pv{tt}")
                        pv = v_ps[tt]
                        kcs = (range(NM) if half is None
                               else range(4 * half, 4 * half + 4))
                        for kc in kcs:
                            nc.tensor.matmul(
                                pv,
                                lhsT=xt[:, kc, tt * 128:(tt + 1) * 128],
                                rhs=wv_sb[:, kc, :],
                                start=(kc == 0), stop=(kc == NM - 1))
                        if half in (1, None):
                            v4 = vsb[tt]
                            nc.vector.tensor_add(
                                out=v4[:, :, 0:DH],
                                in0=pv.rearrange("p (h c) -> p h c", h=HPG),
                                in1=bv4)
                    return f
                if v_chunk_out is not None:
                    v_chunk_out.append(v_chunk)
                    for tt in range(4 * g, 4 * g + 4):
                        items.append(None)  # V items emitted by caller
                    items = [i for i in items if i is not None]
                else:
                    for tt in range(4 * g, 4 * g + 4):
                        items.append(v_chunk(tt))
                return items

            def c_items(qc):
                items = []
                nkt = 4 * qc + 4
                pavs = {}

                def pair_step(h, ktp):
                    o, r = h // 2, (h % 2) * 64
                    def f():
                        qt_h = qt[o][r:r + 64, :]
                        kt_h = kt_[o][r:r + 64, :]
                        if ktp == 0:
                            pavs[h] = pp.tile([VW, QC], F32, tag="pav",
                                              bufs=2, name=f"pav{qc}_{h}")
                        pav = pavs[h]
                        kts = [k for k in (ktp, ktp + 1) if k < nkt]
                        w = 512 * len(kts)
                        ps = pp.tile([128, 1024], F32, tag="ps",
                                     name=f"ps{qc}_{h}_{ktp}")
                        offs = [max(k * 128 - qc * QC, 0) for k in kts]
                        for i, k in enumerate(kts):
                            nc.tensor.matmul(
                                ps[:, i * 512 + offs[i]:(i + 1) * 512],
                                lhsT=kt_h[:, k * 128:(k + 1) * 128],
                                rhs=qt_h[:, qc * QC + offs[i]:(qc + 1) * QC],
                                start=True, stop=True)
                        at = apool.tile([128, 1024], BF16, tag="at",
                                        name=f"at{qc}_{h}_{ktp}")
                        nc.scalar.activation(
                            out=at[:, :w], in_=ps[:, :w],
                            func=mybir.ActivationFunctionType.Exp,
                            scale=1.0 / 8.0)
                        for i, k in enumerate(kts):
                            off = offs[i]
                            if k * 128 - qc * QC >= 0:
                                # causal mask on the diagonal 128-block:
                                # keep q >= k (f >= p), zero the rest
                                blk = at[:, i * 512 + off:i * 512 + off + 128]
                                if tri_engine == 'pool':
                                    nc.gpsimd.affine_select(
                                        out=blk, in_=blk,
                                        compare_op=mybir.AluOpType.is_ge,
                                        fill=0.0, base=0, channel_multiplier=-1,
                                        pattern=[[1, 128]])
                                else:
                                    nc.vector.tensor_mul(out=blk, in0=blk, in1=tri)
                            nc.tensor.matmul(
                                pav[:, off:QC],
                                lhsT=vsb[k][:, h, :],
                                rhs=at[:, i * 512 + off:(i + 1) * 512],
                                start=(k == 0), stop=(k == nkt - 1))
                    return f

                def norm_step(h):
                    o, r = h // 2, (h % 2) * 64
                    def f():
                        pav = pavs[h]
                        rec = rpool.tile([1, QC], BF16, tag="rec",
                                         name=f"rec{qc}_{h}")
                        with nc.allow_low_precision(
                                reason="softmax normalizer bf16; matches "
                                       "kernel-wide bf16 error budget"):
                            nc.vector.reciprocal(out=rec, in_=pav[DH:VW, :])
                        recb = pp.tile([DH, QC], F32, tag="gp",
                                       name=f"recb{qc}_{h}")
                        nc.tensor.matmul(recb, lhsT=ones1, rhs=rec,
                                         start=True, stop=True)
                        rb = rpool.tile([DH, QC], BF16, tag="rb",
                                        name=f"rb{qc}_{h}")
                        nc.vector.tensor_copy(out=rb, in_=recb)
                        nc.vector.tensor_mul(
                            out=ot_[o][r:r + 64, qc * QC:(qc + 1) * QC],
                            in0=pav[0:DH, :], in1=rb)
                    return f

                for h in range(HPG):
                    for ktp in range(0, nkt, 2):
                        items.append(pair_step(h, ktp))
                    items.append(norm_step(h))
                return items

            def d_items(g, drain_eng='dve'):
                items = []

                def out_tile(tt):
                    def f():
                        ob = opool.tile([128, D], BF16, tag="ob", name=f"ob{tt}")
                        for nb in range(2):
                            po = pp.tile([128, 512], F32, tag="gp",
                                         name=f"po{tt}_{nb}")
                            for cb in range(2):
                                nc.tensor.matmul(
                                    po,
                                    lhsT=ot_[cb][:, tt * 128:(tt + 1) * 128],
                                    rhs=wo_sb[:, cb, nb * 512:(nb + 1) * 512],
                                    start=(cb == 0), stop=(cb == 1))
                            if drain_eng == 'act' or (
                                    drain_eng == 'alt' and (tt + nb) % 2 == 1):
                                nc.scalar.copy(
                                    out=ob[:, nb * 512:(nb + 1) * 512], in_=po)
                            else:
                                nc.vector.tensor_copy(
                                    out=ob[:, nb * 512:(nb + 1) * 512], in_=po)
                        nc.sync.dma_start(
                            out=out_d[tt * 128:(tt + 1) * 128, :], in_=ob)
                    return f
                for tt in range(4 * g, 4 * g + 4):
                    items.append(out_tile(tt))
                return items

            # round 0: group 0 projections, optionally merged with heads
            # 0/1 of group-0 attention (hand-ordered so the in-order PE
            # stream reaches the first score matmul once the minimal prefix
            # Qo0/Ko0/V01 is ready)
            if r0merge:
                _vc = []
                a0 = ab_items(0, v_chunk_out=_vc)
                v_chunk0 = _vc[0]
                c0 = c_items(0)
                v0a = [v_chunk0(tt, 0) for tt in range(4)]
                v0b = [v_chunk0(tt, 1) for tt in range(4)]
                seq0 = ([a0[0], a0[2], v0a[0], v0a[1],
                         a0[1], a0[3], v0b[0], v0b[1],
                         c0[0],
                         v0a[2], v0a[3], v0b[2], v0b[3],
                         c0[1], a0[4], c0[3], a0[5], c0[2], a0[6],
                         c0[4], a0[7], c0[5]])
                for f in seq0:
                    f()
                rest0 = c0[6:]
            else:
                for f in ab_items(0):
                    f()
                rest0 = None
            # rounds 1..3: attention(r-1) interleaved with projections(r)
            for r in range(1, NQC):
                prev = (rest0 if (r == 1 and rest0 is not None)
                        else c_items(r - 1))
                for f in interleave(prev, ab_items(r), lead=lead):
                    f()
            # final attention group interleaved with the first 3 groups'
            # output projections (their PE work fills exp() stalls);
            # the trailing group's drains go to ACT, idle once exps finish
            dfill = (d_items(0, drain_eng=ob_drain)
                     + d_items(1, drain_eng=ob_drain)
                     + d_items(2, drain_eng=ob_drain))
            for f in interleave(c_items(NQC - 1), dfill, lead=lead):
                f()
            for f in d_items(NQC - 1, drain_eng='act'):
                f()
    return nc


BUILD_OPTS = dict(tri_engine='dve', ob_drain='dve', lead=0.1, r0merge=False, rbufs=3, obufs=4)


def _get_nc():
    key = str(sorted(BUILD_OPTS.items()))
    if key not in _NC_CACHE:
        _patch_tile_drain()
        _patch_compile_hook()
        _NC_CACHE[key] = build_nc(**BUILD_OPTS)
    return _NC_CACHE[key]


def make_in_maps(inputs):
    x = np.asarray(inputs["x"], dtype=np.float32)
    Wq = np.asarray(inputs["Wq"], dtype=np.float32)
    Wk = np.asarray(inputs["Wk"], dtype=np.float32)
    Wv = np.asarray(inputs["Wv"], dtype=np.float32)
    Wo = np.asarray(inputs["Wo"], dtype=np.float32)
    bq = np.asarray(inputs["bq"], dtype=np.float32)
    bk = np.asarray(inputs["bk"], dtype=np.float32)
    bv = np.asarray(inputs["bv"], dtype=np.float32)

    # x^T per batch, bf16, contiguous: [D, S]
    xT = [np.ascontiguousarray(x[b].T.astype(bf16)) for b in range(B)]

    def w_cols(W, cols):
        # [D, 256] -> SBUF layout [128, 8, 256] flattened to [128, 2048]
        w = W[:, cols].astype(bf16)
        return np.ascontiguousarray(
            w.reshape(NM, 128, OC).transpose(1, 0, 2).reshape(128, NM * OC))

    def w_rows(W, rows):
        # [256, D] -> SBUF layout [128, 2, 1024] flattened to [128, 2048]
        w = W[rows, :].astype(bf16)
        return np.ascontiguousarray(
            w.reshape(2, 128, D).transpose(1, 0, 2).reshape(128, 2 * D))

    # per-head-group tensors are shared by the two batch cores (c and c+4)
    gmaps = []
    for g in range(4):
        cols = slice(g * OC, (g + 1) * OC)
        gmaps.append({
            "wq": w_cols(Wq, cols),
            "wk": w_cols(Wk, cols),
            "wv": w_cols(Wv, cols),
            "wo": w_rows(Wo, cols),
            "bq": np.ascontiguousarray(bq[cols]),
            "bk": np.ascontiguousarray(bk[cols]),
            "bv": np.ascontiguousarray(bv[cols]),
        })
    return [{"xt": xT[c // 4], **gmaps[c % 4]} for c in range(8)]


def combine(results, inputs):
    bo = np.asarray(inputs["bo"], dtype=np.float32)
    out = np.zeros((B, S, D), dtype=np.float32)
    for c in range(8):
        out[c // 4] += results[c]["out"].astype(np.float32)
    out += bo[None, None, :]
    return out


def kernel(**inputs) -> np.ndarray:
    nc = _get_nc()
    in_maps = make_in_maps(inputs)
    res = run_bass_kernel_spmd(nc, in_maps, core_ids=list(range(8)))
    return combine(res.results, inputs)


if __name__ == "__main__":
    import jax
    print(jax.devices())


# revision 28
# speedup vs baseline: 1.0064x; 1.0038x over previous
"""Causal self-attention (B=2, S=2048, D=1024, H=16) on 8 trn2 NeuronCores.

Sharding: batch x head-group. Core c handles batch c//4 and heads
[ (c%4)*4 , (c%4)*4+4 ).  QKV projections are column-sharded, the output
projection row-sharded (Megatron style); each core produces a partial
[S, D] output (bf16) which the host sums per batch.

v2 layout strategy (everything "transposed", all matmul operands bf16):
  x^T   [D, S]   supplied pre-transposed + pre-cast by the host: the kernel
        DMAs it straight into SBUF (no PE transposes, no stage drains).
  Weights arrive pre-arranged so each is a single contiguous [128, 2048]
        bf16 DMA straight into its SBUF operand layout.
  Q^T,K^T [256, S] = W^T x^T  (lhsT = W cols, rhs = x^T chunks), bf16.
  V     [S, 256] = x W  (lhsT = x^T tiles, rhs = Wv), bf16, padded with a
        ones column per head -> AV matmul also produces the softmax
        normalizer l = sum_k exp(s) as an extra output row.
  S^T   [k, q] score chunks in PSUM; exp() applied directly (scores are
        bounded for this problem so no running-max is needed); causal mask =
        skip the fully-masked leading columns in the S/AV matmuls + one
        triangular 0/1 multiply on the diagonal 128-block of the exp output.
  out'^T [65, q] = [V|1]^T A^T accumulated over k tiles in PSUM.
  O^T = out'^T[0:64] * (1/l): 1/l (bf16) is broadcast across partitions with
        a rank-1 PE matmul (ones[1,64]^T @ rec[1,512]) instead of a DRAM
        round trip.
  out   [S, D] partial (bf16) = O^T^T Wo accumulated over 2 feature chunks.
"""

import numpy as np
import ml_dtypes

import concourse.bass as bass
import concourse.mybir as mybir
import concourse.tile as tile
from concourse.bass_utils import run_bass_kernel_spmd

B, S, D = 2, 2048, 1024
HPG, DH = 4, 64            # heads per core, head dim
OC = HPG * DH              # 256 projection cols per core
VW = DH + 1                # V padded with ones column
NT = S // 128              # 16 token tiles
NM = D // 128              # 8 dmodel chunks
QC = 512                   # q chunk width
NQC = S // QC              # 4 q chunks
F32 = mybir.dt.float32
BF16 = mybir.dt.bfloat16
bf16 = ml_dtypes.bfloat16

_NC_CACHE = {}


WAIT_CAP = 1


def _split_waits_bir(bir_json, cap=WAIT_CAP):
    """This container's walrus rejects instructions carrying more than `cap`
    sync waits.  Hoist the excess into standalone same-engine EventSemaphore
    wait ops immediately before the instruction (sequencers execute in
    order, so semantics are identical)."""
    import json as _json

    d = _json.loads(bir_json)
    n_split = 0
    for f in d.get("functions", []):
        for bb in f.get("blocks", []):
            insts = bb.get("instructions", [])
            out = []
            for inst in insts:
                si = inst.get("sync_info")
                ow = (si or {}).get("on_wait") or []
                sem_w = [w for w in ow if w.get("sync_type") == "semaphore"]
                other_w = [w for w in ow if w.get("sync_type") != "semaphore"]
                budget = max(cap - len(other_w), 0)
                if len(sem_w) > budget:
                    keep = sem_w[:budget] if budget else []
                    extra = sem_w[budget:]
                    step = max(cap, 1)
                    for i in range(0, len(extra), step):
                        n_split += 1
                        out.append({
                            "debug": inst.get("debug"),
                            "engine": inst["engine"],
                            "ins": [],
                            "name": f"{inst['name']}_sw{i}",
                            "opcode": "EventSemaphore",
                            "outs": [],
                            "sync_info": {"on_update": [],
                                          "on_wait": extra[i:i + step]},
                        })
                    si["on_wait"] = other_w + keep
                out.append(inst)
            bb["instructions"] = out
    return _json.dumps(d).encode(), n_split


def _patch_compile_hook():
    import concourse.bass_utils as bu
    import concourse.bass2jax as b2j

    orig = bu.compile_bir_kernel
    if getattr(orig, "_split_waits_wrapped", False):
        return

    def wrapped(bir_json, tmpdir, neff_name="file.neff"):
        if isinstance(bir_json, str):
            bir_json = bir_json.encode()
        bir_json, _ = _split_waits_bir(bir_json)
        return orig(bir_json, tmpdir, neff_name)

    wrapped._split_waits_wrapped = True
    bu.compile_bir_kernel = wrapped
    if getattr(b2j, "compile_bir_kernel", None) is orig:
        b2j.compile_bir_kernel = wrapped


def _patch_tile_drain():
    """This container's walrus rejects >2 sync waits on one SP CTRL op; the
    stock Tile exit drain carries one wait per active proc.  Emit separate
    single-wait instructions instead."""
    from concourse.vector_clock import ScopedClock  # noqa: F401

    def _drain_split(self, tick_clock, wait_clock):
        nc = self.nc
        sems = wait_clock.sems.allocated()
        for proc, t in enumerate(list(tick_clock.global_clock)):
            if t <= 0:
                continue
            sem = sems.get(proc)
            if sem is None:
                continue
            nc.sync.wait_ge(sem, t * (16 if sem.name.startswith("DMA") else 1))
        nc.sync.drain()
        nc.all_engine_barrier()
        popped = nc._tile_sem_poison_stack.pop()
        assert popped is self._sem_poison
        nc.clear_and_free_semaphores(list(self.sems.allocated().values()))
        nc.all_engine_barrier()

    tile.TileContext._drain_and_barrier = _drain_split


def _bc(ap, n):
    """Broadcast a [1, ...] DRAM AP across n partitions (step-0 partition)."""
    return bass.AP(tensor=ap.tensor, offset=ap.offset, ap=[[0, n]] + list(ap.ap)[1:])


def build_nc(tri_engine='pool', ob_drain='dve', lead=0.15, r0merge=False, rbufs=2, obufs=3):
    nc = bass.Bass()
    xt_d = nc.dram_tensor("xt", [D, S], BF16, kind="ExternalInput")
    wq_d = nc.dram_tensor("wq", [128, NM * OC], BF16, kind="ExternalInput")
    wk_d = nc.dram_tensor("wk", [128, NM * OC], BF16, kind="ExternalInput")
    wv_d = nc.dram_tensor("wv", [128, NM * OC], BF16, kind="ExternalInput")
    wo_d = nc.dram_tensor("wo", [128, 2 * D], BF16, kind="ExternalInput")
    bq_d = nc.dram_tensor("bq", [OC], F32, kind="ExternalInput")
    bk_d = nc.dram_tensor("bk", [OC], F32, kind="ExternalInput")
    bv_d = nc.dram_tensor("bv", [OC], F32, kind="ExternalInput")
    out_d = nc.dram_tensor("out", [S, D], BF16, kind="ExternalOutput")

    bqr = bq_d.rearrange("(p one) -> p one", one=1)
    bkr = bk_d.rearrange("(p one) -> p one", one=1)
    bvr = bv_d.rearrange("(one c) -> one c", one=1)

    with tile.TileContext(nc) as tc:
        with (
            tc.tile_pool(name="singles", bufs=1) as sing,
            tc.tile_pool(name="persist", bufs=1) as per,
            tc.tile_pool(name="apool", bufs=8) as apool,
            tc.tile_pool(name="rpool", bufs=rbufs) as rpool,
            tc.tile_pool(name="opool", bufs=obufs) as opool,
            tc.tile_pool(name="pp", bufs=2, space="PSUM") as pp,
        ):
            wq_sb = sing.tile([128, NM, OC], BF16, tag="wq")
            wk_sb = sing.tile([128, NM, OC], BF16, tag="wk")
            wv_sb = sing.tile([128, NM, OC], BF16, tag="wv")
            wo_sb = sing.tile([128, 2, D], BF16, tag="wo")
            xt = per.tile([128, NM, S], BF16, tag="xt")

            # Weight DMAs first (small, needed by the first projections), then
            # x^T in S-halves so group-0/1 projections start after ~3 MB of
            # input instead of the full 5.5 MB; projections consume chunk kc
            # as it lands.  Input loads alternate between the two HWDGE rings
            # (qSPDynamicHW via nc.sync, qActDynamicHW via nc.scalar) so
            # descriptor generation for the startup burst runs in parallel.
            rings = [nc.sync, nc.scalar]

            def in_dma(i, out, in_):
                rings[i % 2].dma_start(out=out, in_=in_)

            in_dma(0, wq_sb.rearrange("p c n -> p (c n)"), wq_d[:, :])
            in_dma(1, xt[:, 0, 0:S // 2], xt_d[0:128, 0:S // 2])
            in_dma(0, wk_sb.rearrange("p c n -> p (c n)"), wk_d[:, :])
            in_dma(1, wv_sb.rearrange("p c n -> p (c n)"), wv_d[:, :])
            for kc in range(1, NM):
                in_dma(kc, xt[:, kc, 0:S // 2],
                       xt_d[kc * 128:(kc + 1) * 128, 0:S // 2])

            bq_sb = sing.tile([128, 2], F32, tag="bq")
            bk_sb = sing.tile([128, 2], F32, tag="bk")
            for o in range(2):
                nc.sync.dma_start(out=bq_sb[:, o:o + 1], in_=bqr[o * 128:(o + 1) * 128, :])
                nc.scalar.dma_start(out=bk_sb[:, o:o + 1], in_=bkr[o * 128:(o + 1) * 128, :])
            bv_sb = sing.tile([128, OC], F32, tag="bv")
            nc.sync.dma_start(out=bv_sb, in_=_bc(bvr[0:1, :], 128))
            bv4 = bv_sb.rearrange("p (h c) -> p h c", h=HPG)

            for kc in range(NM):
                in_dma(kc, xt[:, kc, S // 2:S],
                       xt_d[kc * 128:(kc + 1) * 128, S // 2:S])
            nc.sync.dma_start(out=wo_sb.rearrange("p c n -> p (c n)"),
                              in_=wo_d[:, :])

            ones1 = sing.tile([1, DH], BF16, tag="ones1")
            nc.vector.memset(ones1, 1.0)
            if tri_engine == 'dve':
                tri = sing.tile([128, 128], BF16, tag="tri")
                nc.vector.memset(tri, 1.0)
                nc.gpsimd.affine_select(
                    out=tri, in_=tri, compare_op=mybir.AluOpType.is_ge,
                    fill=0.0, base=0, channel_multiplier=-1, pattern=[[1, 128]])

            qt = [per.tile([128, S], BF16, tag=f"qt{o}", name=f"qt{o}") for o in range(2)]
            kt_ = [per.tile([128, S], BF16, tag=f"kt{o}", name=f"kt{o}") for o in range(2)]
            ot_ = [per.tile([128, S], BF16, tag=f"ot{o}", name=f"ot{o}") for o in range(2)]
            vsb = [per.tile([128, HPG, VW], BF16, tag=f"v{t}", name=f"v{t}") for t in range(NT)]
            for t in range(NT):
                nc.gpsimd.memset(vsb[t][:, :, DH:VW], 1.0)

            # Software pipeline over 4 token groups: group g's attention is
            # interleaved with group g+1's projections and the tail groups'
            # output projections so the (in-order) PE stream always has
            # non-attention work to run while ACT evaluates exp().
            def interleave(*lists, lead=0.0):
                # lead > 0 front-loads the FIRST list (attention items) so
                # ACT gets score tiles to exp() right at the round start.
                import heapq
                h, out = [], []
                for li, L in enumerate(lists):
                    if L:
                        start = -lead if li == 0 else 0.0
                        heapq.heappush(h, (start, li, 0))
                while h:
                    pos, li, idx = heapq.heappop(h)
                    out.append(lists[li][idx])
                    if idx + 1 < len(lists[li]):
                        heapq.heappush(h, (pos + 1.0 / len(lists[li]), li, idx + 1))
                return out

            def windowed(*windows):
                # windows: (items, start, end) — item i of a list sits at
                # position start + (i + 1) * (end - start) / len; merged by
                # position (stable for equal positions by list order).
                entries = []
                for li, (L, s, e) in enumerate(windows):
                    n = len(L)
                    for i, it in enumerate(L):
                        entries.append((s + (i + 1) * (e - s) / n, li, i, it))
                entries.sort(key=lambda t: (t[0], t[1], t[2]))
                return [t[3] for t in entries]

            def ab_items(g, v_chunk_out=None):
                items = []
                qk_ps = {}

                def qk_chunk(wsb, bsb, dst, o, half):
                    def f():
                        if half == 0:
                            qk_ps[(id(wsb), o)] = pp.tile(
                                [128, QC], F32, tag="gp", name=f"qk{g}_{o}")
                        ps = qk_ps[(id(wsb), o)]
                        for kc in range(4 * half, 4 * half + 4):
                            nc.tensor.matmul(
                                ps,
                                lhsT=wsb[:, kc, o * 128:(o + 1) * 128],
                                rhs=xt[:, kc, g * QC:(g + 1) * QC],
                                start=(kc == 0), stop=(kc == NM - 1))
                        if half == 1:
                            nc.vector.tensor_scalar_add(
                                out=dst[o][:, g * QC:(g + 1) * QC],
                                in0=ps, scalar1=bsb[:, o:o + 1])
                    return f
                # o-major: Q/K for o=0 complete first so heads 0/1 scores
                # (and their exp stream) can start before o=1 / V work runs
                for o in range(2):
                    for wsb, bsb, dst in ((wq_sb, bq_sb, qt), (wk_sb, bk_sb, kt_)):
                        for half in range(2):
                            items.append(qk_chunk(wsb, bsb, dst, o, half))

                v_ps = {}

                def v_chunk(tt, half=None):
                    def f():
                        if half in (0, None):
                            v_ps[tt] = pp.tile([128, OC], F32, tag="gp",
                                               name=f"pv{tt}")
                        pv = v_ps[tt]
                        kcs = (range(NM) if half is None
                               else range(4 * half, 4 * half + 4))
                        for kc in kcs:
                            nc.tensor.matmul(
                                pv,
                                lhsT=xt[:, kc, tt * 128:(tt + 1) * 128],
                                rhs=wv_sb[:, kc, :],
                                start=(kc == 0), stop=(kc == NM - 1))
                        if half in (1, None):
                            v4 = vsb[tt]
                            nc.vector.tensor_add(
                                out=v4[:, :, 0:DH],
                                in0=pv.rearrange("p (h c) -> p h c", h=HPG),
                                in1=bv4)
                    return f
                if v_chunk_out is not None:
                    v_chunk_out.append(v_chunk)
                    for tt in range(4 * g, 4 * g + 4):
                        items.append(None)  # V items emitted by caller
                    items = [i for i in items if i is not None]
                else:
                    for tt in range(4 * g, 4 * g + 4):
                        items.append(v_chunk(tt))
                return items

            def c_items(qc):
                items = []
                nkt = 4 * qc + 4
                pavs = {}

                def pair_step(h, ktp):
                    o, r = h // 2, (h % 2) * 64
                    def f():
                        qt_h = qt[o][r:r + 64, :]
                        kt_h = kt_[o][r:r + 64, :]
                        if ktp == 0:
                            pavs[h] = pp.tile([VW, QC], F32, tag="pav",
                                              bufs=2, name=f"pav{qc}_{h}")
                        pav = pavs[h]
                        kts = [k for k in (ktp, ktp + 1) if k < nkt]
                        w = 512 * len(kts)
                        ps = pp.tile([128, 1024], F32, tag="ps",
                                     name=f"ps{qc}_{h}_{ktp}")
                        offs = [max(k * 128 - qc * QC, 0) for k in kts]
                        for i, k in enumerate(kts):
                            nc.tensor.matmul(
                                ps[:, i * 512 + offs[i]:(i + 1) * 512],
                                lhsT=kt_h[:, k * 128:(k + 1) * 128],
                                rhs=qt_h[:, qc * QC + offs[i]:(qc + 1) * QC],
                                start=True, stop=True)
                        at = apool.tile([128, 1024], BF16, tag="at",
                                        name=f"at{qc}_{h}_{ktp}")
                        nc.scalar.activation(
                            out=at[:, :w], in_=ps[:, :w],
                            func=mybir.ActivationFunctionType.Exp,
                            scale=1.0 / 8.0)
                        for i, k in enumerate(kts):
                            off = offs[i]
                            if k * 128 - qc * QC >= 0:
                                # causal mask on the diagonal 128-block:
                                # keep q >= k (f >= p), zero the rest
                                blk = at[:, i * 512 + off:i * 512 + off + 128]
                                if tri_engine == 'pool':
                                    nc.gpsimd.affine_select(
                                        out=blk, in_=blk,
                                        compare_op=mybir.AluOpType.is_ge,
                                        fill=0.0, base=0, channel_multiplier=-1,
                                        pattern=[[1, 128]])
                                else:
                                    nc.vector.tensor_mul(out=blk, in0=blk, in1=tri)
                            nc.tensor.matmul(
                                pav[:, off:QC],
                                lhsT=vsb[k][:, h, :],
                                rhs=at[:, i * 512 + off:(i + 1) * 512],
                                start=(k == 0), stop=(k == nkt - 1))
                    return f

                def norm_step(h):
                    o, r = h // 2, (h % 2) * 64
                    def f():
                        pav = pavs[h]
                        rec = rpool.tile([1, QC], BF16, tag="rec",
                                         name=f"rec{qc}_{h}")
                        with nc.allow_low_precision(
                                reason="softmax normalizer bf16; matches "
                                       "kernel-wide bf16 error budget"):
                            nc.vector.reciprocal(out=rec, in_=pav[DH:VW, :])
                        recb = pp.tile([DH, QC], F32, tag="gp",
                                       name=f"recb{qc}_{h}")
                        nc.tensor.matmul(recb, lhsT=ones1, rhs=rec,
                                         start=True, stop=True)
                        rb = rpool.tile([DH, QC], BF16, tag="rb",
                                        name=f"rb{qc}_{h}")
                        nc.vector.tensor_copy(out=rb, in_=recb)
                        nc.vector.tensor_mul(
                            out=ot_[o][r:r + 64, qc * QC:(qc + 1) * QC],
                            in0=pav[0:DH, :], in1=rb)
                    return f

                for h in range(HPG):
                    for ktp in range(0, nkt, 2):
                        items.append(pair_step(h, ktp))
                    items.append(norm_step(h))
                return items

            def d_items(g, drain_eng='dve'):
                items = []

                def out_tile(tt):
                    def f():
                        ob = opool.tile([128, D], BF16, tag="ob", name=f"ob{tt}")
                        for nb in range(2):
                            po = pp.tile([128, 512], F32, tag="gp",
                                         name=f"po{tt}_{nb}")
                            for cb in range(2):
                                nc.tensor.matmul(
                                    po,
                                    lhsT=ot_[cb][:, tt * 128:(tt + 1) * 128],
                                    rhs=wo_sb[:, cb, nb * 512:(nb + 1) * 512],
                                    start=(cb == 0), stop=(cb == 1))
                            if drain_eng == 'act' or (
                                    drain_eng == 'alt' and (tt + nb) % 2 == 1):
                                nc.scalar.copy(
                                    out=ob[:, nb * 512:(nb + 1) * 512], in_=po)
                            else:
                                nc.vector.tensor_copy(
                                    out=ob[:, nb * 512:(nb + 1) * 512], in_=po)
                            if drain_eng == 'act':
                                # trailing group: ship each half as soon as it
                                # drains so the exit barrier waits on a
                                # smaller, earlier-started final DMA
                                nc.sync.dma_start(
                                    out=out_d[tt * 128:(tt + 1) * 128,
                                              nb * 512:(nb + 1) * 512],
                                    in_=ob[:, nb * 512:(nb + 1) * 512])
                        if drain_eng != 'act':
                            nc.sync.dma_start(
                                out=out_d[tt * 128:(tt + 1) * 128, :], in_=ob)
                    return f
                for tt in range(4 * g, 4 * g + 4):
                    items.append(out_tile(tt))
                return items

            # round 0: group 0 projections, optionally merged with heads
            # 0/1 of group-0 attention (hand-ordered so the in-order PE
            # stream reaches the first score matmul once the minimal prefix
            # Qo0/Ko0/V01 is ready)
            if r0merge:
                _vc = []
                a0 = ab_items(0, v_chunk_out=_vc)
                v_chunk0 = _vc[0]
                c0 = c_items(0)
                v0a = [v_chunk0(tt, 0) for tt in range(4)]
                v0b = [v_chunk0(tt, 1) for tt in range(4)]
                seq0 = ([a0[0], a0[2], v0a[0], v0a[1],
                         a0[1], a0[3], v0b[0], v0b[1],
                         c0[0],
                         v0a[2], v0a[3], v0b[2], v0b[3],
                         c0[1], a0[4], c0[3], a0[5], c0[2], a0[6],
                         c0[4], a0[7], c0[5]])
                for f in seq0:
                    f()
                rest0 = c0[6:]
            else:
                for f in ab_items(0):
                    f()
                rest0 = None
            # rounds 1..3: attention(r-1) interleaved with projections(r)
            for r in range(1, NQC):
                prev = (rest0 if (r == 1 and rest0 is not None)
                        else c_items(r - 1))
                for f in interleave(prev, ab_items(r), lead=lead):
                    f()
            # final attention group interleaved with the first 3 groups'
            # output projections (their PE work fills exp() stalls);
            # the trailing group's drains go to ACT, idle once exps finish
            dfill = (d_items(0, drain_eng=ob_drain)
                     + d_items(1, drain_eng=ob_drain)
                     + d_items(2, drain_eng=ob_drain))
            for f in interleave(c_items(NQC - 1), dfill, lead=lead):
                f()
            for f in d_items(NQC - 1, drain_eng='act'):
                f()
    return nc


BUILD_OPTS = dict(tri_engine='dve', ob_drain='dve', lead=0.1, r0merge=False, rbufs=3, obufs=4)


def _get_nc():
    key = str(sorted(BUILD_OPTS.items()))
    if key not in _NC_CACHE:
        _patch_tile_drain()
        _patch_compile_hook()
        _NC_CACHE[key] = build_nc(**BUILD_OPTS)
    return _NC_CACHE[key]


def make_in_maps(inputs):
    x = np.asarray(inputs["x"], dtype=np.float32)
    Wq = np.asarray(inputs["Wq"], dtype=np.float32)
    Wk = np.asarray(inputs["Wk"], dtype=np.float32)
    Wv = np.asarray(inputs["Wv"], dtype=np.float32)
    Wo = np.asarray(inputs["Wo"], dtype=np.float32)
    bq = np.asarray(inputs["bq"], dtype=np.float32)
    bk = np.asarray(inputs["bk"], dtype=np.float32)
    bv = np.asarray(inputs["bv"], dtype=np.float32)

    # x^T per batch, bf16, contiguous: [D, S]
    xT = [np.ascontiguousarray(x[b].T.astype(bf16)) for b in range(B)]

    def w_cols(W, cols):
        # [D, 256] -> SBUF layout [128, 8, 256] flattened to [128, 2048]
        w = W[:, cols].astype(bf16)
        return np.ascontiguousarray(
            w.reshape(NM, 128, OC).transpose(1, 0, 2).reshape(128, NM * OC))

    def w_rows(W, rows):
        # [256, D] -> SBUF layout [128, 2, 1024] flattened to [128, 2048]
        w = W[rows, :].astype(bf16)
        return np.ascontiguousarray(
            w.reshape(2, 128, D).transpose(1, 0, 2).reshape(128, 2 * D))

    # per-head-group tensors are shared by the two batch cores (c and c+4)
    gmaps = []
    for g in range(4):
        cols = slice(g * OC, (g + 1) * OC)
        gmaps.append({
            "wq": w_cols(Wq, cols),
            "wk": w_cols(Wk, cols),
            "wv": w_cols(Wv, cols),
            "wo": w_rows(Wo, cols),
            "bq": np.ascontiguousarray(bq[cols]),
            "bk": np.ascontiguousarray(bk[cols]),
            "bv": np.ascontiguousarray(bv[cols]),
        })
    return [{"xt": xT[c // 4], **gmaps[c % 4]} for c in range(8)]


def combine(results, inputs):
    bo = np.asarray(inputs["bo"], dtype=np.float32)
    out = np.zeros((B, S, D), dtype=np.float32)
    for c in range(8):
        out[c // 4] += results[c]["out"].astype(np.float32)
    out += bo[None, None, :]
    return out


def kernel(**inputs) -> np.ndarray:
    nc = _get_nc()
    in_maps = make_in_maps(inputs)
    res = run_bass_kernel_spmd(nc, in_maps, core_ids=list(range(8)))
    return combine(res.results, inputs)


if __name__ == "__main__":
    import jax
    print(jax.devices())


# revision 29
# speedup vs baseline: 1.0205x; 1.0140x over previous
"""Causal self-attention (B=2, S=2048, D=1024, H=16) on 8 trn2 NeuronCores.

Sharding: batch x head-group. Core c handles batch c//4 and heads
[ (c%4)*4 , (c%4)*4+4 ).  QKV projections are column-sharded, the output
projection row-sharded (Megatron style); each core produces a partial
[S, D] output (bf16) which the host sums per batch.

v2 layout strategy (everything "transposed", all matmul operands bf16):
  x^T   [D, S]   supplied pre-transposed + pre-cast by the host: the kernel
        DMAs it straight into SBUF (no PE transposes, no stage drains).
  Weights arrive pre-arranged so each is a single contiguous [128, 2048]
        bf16 DMA straight into its SBUF operand layout.
  Q^T,K^T [256, S] = W^T x^T  (lhsT = W cols, rhs = x^T chunks), bf16.
  V     [S, 256] = x W  (lhsT = x^T tiles, rhs = Wv), bf16, padded with a
        ones column per head -> AV matmul also produces the softmax
        normalizer l = sum_k exp(s) as an extra output row.
  S^T   [k, q] score chunks in PSUM; exp() applied directly (scores are
        bounded for this problem so no running-max is needed); causal mask =
        skip the fully-masked leading columns in the S/AV matmuls + one
        triangular 0/1 multiply on the diagonal 128-block of the exp output.
  out'^T [65, q] = [V|1]^T A^T accumulated over k tiles in PSUM.
  O^T = out'^T[0:64] * (1/l): 1/l (bf16) is broadcast across partitions with
        a rank-1 PE matmul (ones[1,64]^T @ rec[1,512]) instead of a DRAM
        round trip.
  out   [S, D] partial (bf16) = O^T^T Wo accumulated over 2 feature chunks.
"""

import numpy as np
import ml_dtypes

import concourse.bass as bass
import concourse.mybir as mybir
import concourse.tile as tile
from concourse.bass_utils import run_bass_kernel_spmd

B, S, D = 2, 2048, 1024
HPG, DH = 4, 64            # heads per core, head dim
OC = HPG * DH              # 256 projection cols per core
VW = DH + 1                # V padded with ones column
NT = S // 128              # 16 token tiles
NM = D // 128              # 8 dmodel chunks
QC = 512                   # q chunk width
NQC = S // QC              # 4 q chunks
F32 = mybir.dt.float32
BF16 = mybir.dt.bfloat16
bf16 = ml_dtypes.bfloat16

_NC_CACHE = {}


WAIT_CAP = 1


def _split_waits_bir(bir_json, cap=WAIT_CAP):
    """This container's walrus rejects instructions carrying more than `cap`
    sync waits.  Hoist the excess into standalone same-engine EventSemaphore
    wait ops immediately before the instruction (sequencers execute in
    order, so semantics are identical)."""
    import json as _json

    d = _json.loads(bir_json)
    n_split = 0
    for f in d.get("functions", []):
        for bb in f.get("blocks", []):
            insts = bb.get("instructions", [])
            out = []
            for inst in insts:
                si = inst.get("sync_info")
                ow = (si or {}).get("on_wait") or []
                sem_w = [w for w in ow if w.get("sync_type") == "semaphore"]
                other_w = [w for w in ow if w.get("sync_type") != "semaphore"]
                budget = max(cap - len(other_w), 0)
                if len(sem_w) > budget:
                    keep = sem_w[:budget] if budget else []
                    extra = sem_w[budget:]
                    step = max(cap, 1)
                    for i in range(0, len(extra), step):
                        n_split += 1
                        out.append({
                            "debug": inst.get("debug"),
                            "engine": inst["engine"],
                            "ins": [],
                            "name": f"{inst['name']}_sw{i}",
                            "opcode": "EventSemaphore",
                            "outs": [],
                            "sync_info": {"on_update": [],
                                          "on_wait": extra[i:i + step]},
                        })
                    si["on_wait"] = other_w + keep
                out.append(inst)
            bb["instructions"] = out
    return _json.dumps(d).encode(), n_split


def _patch_compile_hook():
    import concourse.bass_utils as bu
    import concourse.bass2jax as b2j

    orig = bu.compile_bir_kernel
    if getattr(orig, "_split_waits_wrapped", False):
        return

    def wrapped(bir_json, tmpdir, neff_name="file.neff"):
        if isinstance(bir_json, str):
            bir_json = bir_json.encode()
        bir_json, _ = _split_waits_bir(bir_json)
        return orig(bir_json, tmpdir, neff_name)

    wrapped._split_waits_wrapped = True
    bu.compile_bir_kernel = wrapped
    if getattr(b2j, "compile_bir_kernel", None) is orig:
        b2j.compile_bir_kernel = wrapped


def _patch_tile_drain():
    """This container's walrus rejects >2 sync waits on one SP CTRL op; the
    stock Tile exit drain carries one wait per active proc.  Emit separate
    single-wait instructions instead."""
    from concourse.vector_clock import ScopedClock  # noqa: F401

    def _drain_split(self, tick_clock, wait_clock):
        nc = self.nc
        sems = wait_clock.sems.allocated()
        for proc, t in enumerate(list(tick_clock.global_clock)):
            if t <= 0:
                continue
            sem = sems.get(proc)
            if sem is None:
                continue
            nc.sync.wait_ge(sem, t * (16 if sem.name.startswith("DMA") else 1))
        nc.sync.drain()
        nc.all_engine_barrier()
        popped = nc._tile_sem_poison_stack.pop()
        assert popped is self._sem_poison
        nc.clear_and_free_semaphores(list(self.sems.allocated().values()))
        nc.all_engine_barrier()

    tile.TileContext._drain_and_barrier = _drain_split


def _bc(ap, n):
    """Broadcast a [1, ...] DRAM AP across n partitions (step-0 partition)."""
    return bass.AP(tensor=ap.tensor, offset=ap.offset, ap=[[0, n]] + list(ap.ap)[1:])


def build_nc(tri_engine='pool', ob_drain='dve', lead=0.15, r0merge=False, rbufs=2, obufs=3):
    nc = bass.Bass()
    xt_d = nc.dram_tensor("xt", [D, S], BF16, kind="ExternalInput")
    wq_d = nc.dram_tensor("wq", [128, NM * OC], BF16, kind="ExternalInput")
    wk_d = nc.dram_tensor("wk", [128, NM * OC], BF16, kind="ExternalInput")
    wv_d = nc.dram_tensor("wv", [128, NM * OC], BF16, kind="ExternalInput")
    wo_d = nc.dram_tensor("wo", [128, 2 * D], BF16, kind="ExternalInput")
    bq_d = nc.dram_tensor("bq", [OC], F32, kind="ExternalInput")
    bk_d = nc.dram_tensor("bk", [OC], F32, kind="ExternalInput")
    bv_d = nc.dram_tensor("bv", [OC], F32, kind="ExternalInput")
    out_d = nc.dram_tensor("out", [S, D], BF16, kind="ExternalOutput")

    bqr = bq_d.rearrange("(p one) -> p one", one=1)
    bkr = bk_d.rearrange("(p one) -> p one", one=1)
    bvr = bv_d.rearrange("(one c) -> one c", one=1)

    with tile.TileContext(nc) as tc:
        with (
            tc.tile_pool(name="singles", bufs=1) as sing,
            tc.tile_pool(name="persist", bufs=1) as per,
            tc.tile_pool(name="apool", bufs=8) as apool,
            tc.tile_pool(name="rpool", bufs=rbufs) as rpool,
            tc.tile_pool(name="opool", bufs=obufs) as opool,
            tc.tile_pool(name="pp", bufs=2, space="PSUM") as pp,
        ):
            wq_sb = sing.tile([128, NM, OC], BF16, tag="wq")
            wk_sb = sing.tile([128, NM, OC], BF16, tag="wk")
            wv_sb = sing.tile([128, NM, OC], BF16, tag="wv")
            wo_sb = sing.tile([128, 2, D], BF16, tag="wo")
            xt = per.tile([128, NM, S], BF16, tag="xt")

            # Weight DMAs first (small, needed by the first projections), then
            # x^T in S-halves so group-0/1 projections start after ~3 MB of
            # input instead of the full 5.5 MB; projections consume chunk kc
            # as it lands.  Input loads alternate between the two HWDGE rings
            # (qSPDynamicHW via nc.sync, qActDynamicHW via nc.scalar) so
            # descriptor generation for the startup burst runs in parallel.
            rings = [nc.sync, nc.scalar]

            def in_dma(i, out, in_):
                rings[i % 2].dma_start(out=out, in_=in_)

            in_dma(0, wq_sb.rearrange("p c n -> p (c n)"), wq_d[:, :])
            in_dma(1, xt[:, 0, 0:S // 2], xt_d[0:128, 0:S // 2])
            in_dma(0, wk_sb.rearrange("p c n -> p (c n)"), wk_d[:, :])
            in_dma(1, wv_sb.rearrange("p c n -> p (c n)"), wv_d[:, :])
            for kc in range(1, NM):
                in_dma(kc, xt[:, kc, 0:S // 2],
                       xt_d[kc * 128:(kc + 1) * 128, 0:S // 2])

            bq_sb = sing.tile([128, 2], F32, tag="bq")
            bk_sb = sing.tile([128, 2], F32, tag="bk")
            for o in range(2):
                nc.sync.dma_start(out=bq_sb[:, o:o + 1], in_=bqr[o * 128:(o + 1) * 128, :])
                nc.scalar.dma_start(out=bk_sb[:, o:o + 1], in_=bkr[o * 128:(o + 1) * 128, :])
            bv_sb = sing.tile([128, OC], F32, tag="bv")
            nc.sync.dma_start(out=bv_sb, in_=_bc(bvr[0:1, :], 128))
            bv4 = bv_sb.rearrange("p (h c) -> p h c", h=HPG)

            for kc in range(NM):
                in_dma(kc, xt[:, kc, S // 2:S],
                       xt_d[kc * 128:(kc + 1) * 128, S // 2:S])
            nc.sync.dma_start(out=wo_sb.rearrange("p c n -> p (c n)"),
                              in_=wo_d[:, :])

            ones1 = sing.tile([1, DH], BF16, tag="ones1")
            nc.vector.memset(ones1, 1.0)
            if tri_engine == 'dve':
                tri = sing.tile([128, 128], BF16, tag="tri")
                nc.vector.memset(tri, 1.0)
                nc.gpsimd.affine_select(
                    out=tri, in_=tri, compare_op=mybir.AluOpType.is_ge,
                    fill=0.0, base=0, channel_multiplier=-1, pattern=[[1, 128]])

            qt = [per.tile([128, S], BF16, tag=f"qt{o}", name=f"qt{o}") for o in range(2)]
            kt_ = [per.tile([128, S], BF16, tag=f"kt{o}", name=f"kt{o}") for o in range(2)]
            ot_ = [per.tile([128, S], BF16, tag=f"ot{o}", name=f"ot{o}") for o in range(2)]
            vsb = [per.tile([128, HPG, VW], BF16, tag=f"v{t}", name=f"v{t}") for t in range(NT)]
            for t in range(NT):
                nc.gpsimd.memset(vsb[t][:, :, DH:VW], 1.0)

            # Software pipeline over 4 token groups: group g's attention is
            # interleaved with group g+1's projections and the tail groups'
            # output projections so the (in-order) PE stream always has
            # non-attention work to run while ACT evaluates exp().
            def interleave(*lists, lead=0.0):
                # lead > 0 front-loads the FIRST list (attention items) so
                # ACT gets score tiles to exp() right at the round start.
                import heapq
                h, out = [], []
                for li, L in enumerate(lists):
                    if L:
                        start = -lead if li == 0 else 0.0
                        heapq.heappush(h, (start, li, 0))
                while h:
                    pos, li, idx = heapq.heappop(h)
                    out.append(lists[li][idx])
                    if idx + 1 < len(lists[li]):
                        heapq.heappush(h, (pos + 1.0 / len(lists[li]), li, idx + 1))
                return out

            def windowed(*windows):
                # windows: (items, start, end) — item i of a list sits at
                # position start + (i + 1) * (end - start) / len; merged by
                # position (stable for equal positions by list order).
                entries = []
                for li, (L, s, e) in enumerate(windows):
                    n = len(L)
                    for i, it in enumerate(L):
                        entries.append((s + (i + 1) * (e - s) / n, li, i, it))
                entries.sort(key=lambda t: (t[0], t[1], t[2]))
                return [t[3] for t in entries]

            def ab_items(g, v_chunk_out=None):
                items = []
                qk_ps = {}

                def qk_chunk(wsb, bsb, dst, o, half):
                    def f():
                        if half == 0:
                            qk_ps[(id(wsb), o)] = pp.tile(
                                [128, QC], F32, tag="gp", name=f"qk{g}_{o}")
                        ps = qk_ps[(id(wsb), o)]
                        for kc in range(4 * half, 4 * half + 4):
                            nc.tensor.matmul(
                                ps,
                                lhsT=wsb[:, kc, o * 128:(o + 1) * 128],
                                rhs=xt[:, kc, g * QC:(g + 1) * QC],
                                start=(kc == 0), stop=(kc == NM - 1))
                        if half == 1:
                            nc.vector.tensor_scalar_add(
                                out=dst[o][:, g * QC:(g + 1) * QC],
                                in0=ps, scalar1=bsb[:, o:o + 1])
                    return f
                # o-major: Q/K for o=0 complete first so heads 0/1 scores
                # (and their exp stream) can start before o=1 / V work runs
                for o in range(2):
                    for wsb, bsb, dst in ((wq_sb, bq_sb, qt), (wk_sb, bk_sb, kt_)):
                        for half in range(2):
                            items.append(qk_chunk(wsb, bsb, dst, o, half))

                v_ps = {}

                def v_chunk(tt, half=None):
                    def f():
                        if half in (0, None):
                            v_ps[tt] = pp.tile([128, OC], F32, tag="gp",
                                               name=f"pv{tt}")
                        pv = v_ps[tt]
                        kcs = (range(NM) if half is None
                               else range(4 * half, 4 * half + 4))
                        for kc in kcs:
                            nc.tensor.matmul(
                                pv,
                                lhsT=xt[:, kc, tt * 128:(tt + 1) * 128],
                                rhs=wv_sb[:, kc, :],
                                start=(kc == 0), stop=(kc == NM - 1))
                        if half in (1, None):
                            v4 = vsb[tt]
                            nc.vector.tensor_add(
                                out=v4[:, :, 0:DH],
                                in0=pv.rearrange("p (h c) -> p h c", h=HPG),
                                in1=bv4)
                    return f
                if v_chunk_out is not None:
                    v_chunk_out.append(v_chunk)
                    for tt in range(4 * g, 4 * g + 4):
                        items.append(None)  # V items emitted by caller
                    items = [i for i in items if i is not None]
                else:
                    for tt in range(4 * g, 4 * g + 4):
                        items.append(v_chunk(tt))
                return items

            def c_items(qc):
                items = []
                nkt = 4 * qc + 4
                pavs = {}

                def pair_step(h, ktp):
                    o, r = h // 2, (h % 2) * 64
                    def f():
                        qt_h = qt[o][r:r + 64, :]
                        kt_h = kt_[o][r:r + 64, :]
                        if ktp == 0:
                            pavs[h] = pp.tile([VW, QC], F32, tag="pav",
                                              bufs=2, name=f"pav{qc}_{h}")
                        pav = pavs[h]
                        kts = [k for k in (ktp, ktp + 1) if k < nkt]
                        w = 512 * len(kts)
                        ps = pp.tile([128, 1024], F32, tag="ps",
                                     name=f"ps{qc}_{h}_{ktp}")
                        offs = [max(k * 128 - qc * QC, 0) for k in kts]
                        for i, k in enumerate(kts):
                            nc.tensor.matmul(
                                ps[:, i * 512 + offs[i]:(i + 1) * 512],
                                lhsT=kt_h[:, k * 128:(k + 1) * 128],
                                rhs=qt_h[:, qc * QC + offs[i]:(qc + 1) * QC],
                                start=True, stop=True)
                        at = apool.tile([128, 1024], BF16, tag="at",
                                        name=f"at{qc}_{h}_{ktp}")
                        # start at the first column the AV matmuls read —
                        # columns below offs[0] are causally dead
                        nc.scalar.activation(
                            out=at[:, offs[0]:w], in_=ps[:, offs[0]:w],
                            func=mybir.ActivationFunctionType.Exp,
                            scale=1.0 / 8.0)
                        for i, k in enumerate(kts):
                            off = offs[i]
                            if k * 128 - qc * QC >= 0:
                                # causal mask on the diagonal 128-block:
                                # keep q >= k (f >= p), zero the rest
                                blk = at[:, i * 512 + off:i * 512 + off + 128]
                                if tri_engine == 'pool':
                                    nc.gpsimd.affine_select(
                                        out=blk, in_=blk,
                                        compare_op=mybir.AluOpType.is_ge,
                                        fill=0.0, base=0, channel_multiplier=-1,
                                        pattern=[[1, 128]])
                                else:
                                    nc.vector.tensor_mul(out=blk, in0=blk, in1=tri)
                            nc.tensor.matmul(
                                pav[:, off:QC],
                                lhsT=vsb[k][:, h, :],
                                rhs=at[:, i * 512 + off:(i + 1) * 512],
                                start=(k == 0), stop=(k == nkt - 1))
                    return f

                def norm_step(h):
                    o, r = h // 2, (h % 2) * 64
                    def f():
                        pav = pavs[h]
                        rec = rpool.tile([1, QC], BF16, tag="rec",
                                         name=f"rec{qc}_{h}")
                        with nc.allow_low_precision(
                                reason="softmax normalizer bf16; matches "
                                       "kernel-wide bf16 error budget"):
                            nc.vector.reciprocal(out=rec, in_=pav[DH:VW, :])
                        recb = pp.tile([DH, QC], F32, tag="gp",
                                       name=f"recb{qc}_{h}")
                        nc.tensor.matmul(recb, lhsT=ones1, rhs=rec,
                                         start=True, stop=True)
                        rb = rpool.tile([DH, QC], BF16, tag="rb",
                                        name=f"rb{qc}_{h}")
                        nc.vector.tensor_copy(out=rb, in_=recb)
                        nc.vector.tensor_mul(
                            out=ot_[o][r:r + 64, qc * QC:(qc + 1) * QC],
                            in0=pav[0:DH, :], in1=rb)
                    return f

                for h in range(HPG):
                    for ktp in range(0, nkt, 2):
                        items.append(pair_step(h, ktp))
                    items.append(norm_step(h))
                return items

            def d_items(g, drain_eng='dve'):
                items = []

                def out_tile(tt):
                    def f():
                        ob = opool.tile([128, D], BF16, tag="ob", name=f"ob{tt}")
                        for nb in range(2):
                            po = pp.tile([128, 512], F32, tag="gp",
                                         name=f"po{tt}_{nb}")
                            for cb in range(2):
                                nc.tensor.matmul(
                                    po,
                                    lhsT=ot_[cb][:, tt * 128:(tt + 1) * 128],
                                    rhs=wo_sb[:, cb, nb * 512:(nb + 1) * 512],
                                    start=(cb == 0), stop=(cb == 1))
                            if drain_eng == 'act' or (
                                    drain_eng == 'alt' and (tt + nb) % 2 == 1):
                                nc.scalar.copy(
                                    out=ob[:, nb * 512:(nb + 1) * 512], in_=po)
                            else:
                                nc.vector.tensor_copy(
                                    out=ob[:, nb * 512:(nb + 1) * 512], in_=po)
                            if drain_eng == 'act':
                                # trailing group: ship each half as soon as it
                                # drains so the exit barrier waits on a
                                # smaller, earlier-started final DMA
                                nc.sync.dma_start(
                                    out=out_d[tt * 128:(tt + 1) * 128,
                                              nb * 512:(nb + 1) * 512],
                                    in_=ob[:, nb * 512:(nb + 1) * 512])
                        if drain_eng != 'act':
                            nc.sync.dma_start(
                                out=out_d[tt * 128:(tt + 1) * 128, :], in_=ob)
                    return f
                for tt in range(4 * g, 4 * g + 4):
                    items.append(out_tile(tt))
                return items

            # round 0: group 0 projections, optionally merged with heads
            # 0/1 of group-0 attention (hand-ordered so the in-order PE
            # stream reaches the first score matmul once the minimal prefix
            # Qo0/Ko0/V01 is ready)
            if r0merge:
                _vc = []
                a0 = ab_items(0, v_chunk_out=_vc)
                v_chunk0 = _vc[0]
                c0 = c_items(0)
                v0a = [v_chunk0(tt, 0) for tt in range(4)]
                v0b = [v_chunk0(tt, 1) for tt in range(4)]
                seq0 = ([a0[0], a0[2], v0a[0], v0a[1],
                         a0[1], a0[3], v0b[0], v0b[1],
                         c0[0],
                         v0a[2], v0a[3], v0b[2], v0b[3],
                         c0[1], a0[4], c0[3], a0[5], c0[2], a0[6],
                         c0[4], a0[7], c0[5]])
                for f in seq0:
                    f()
                rest0 = c0[6:]
            else:
                for f in ab_items(0):
                    f()
                rest0 = None
            # rounds 1..3: attention(r-1) interleaved with projections(r)
            for r in range(1, NQC):
                prev = (rest0 if (r == 1 and rest0 is not None)
                        else c_items(r - 1))
                for f in interleave(prev, ab_items(r), lead=lead):
                    f()
            # final attention group interleaved with the first 3 groups'
            # output projections (their PE work fills exp() stalls);
            # the trailing group's drains go to ACT, idle once exps finish
            dfill = (d_items(0, drain_eng=ob_drain)
                     + d_items(1, drain_eng=ob_drain)
                     + d_items(2, drain_eng=ob_drain))
            for f in interleave(c_items(NQC - 1), dfill, lead=lead):
                f()
            for f in d_items(NQC - 1, drain_eng='act'):
                f()
    return nc


BUILD_OPTS = dict(tri_engine='dve', ob_drain='dve', lead=0.1, r0merge=False, rbufs=3, obufs=4)


def _get_nc():
    key = str(sorted(BUILD_OPTS.items()))
    if key not in _NC_CACHE:
        _patch_tile_drain()
        _patch_compile_hook()
        _NC_CACHE[key] = build_nc(**BUILD_OPTS)
    return _NC_CACHE[key]


def make_in_maps(inputs):
    x = np.asarray(inputs["x"], dtype=np.float32)
    Wq = np.asarray(inputs["Wq"], dtype=np.float32)
    Wk = np.asarray(inputs["Wk"], dtype=np.float32)
    Wv = np.asarray(inputs["Wv"], dtype=np.float32)
    Wo = np.asarray(inputs["Wo"], dtype=np.float32)
    bq = np.asarray(inputs["bq"], dtype=np.float32)
    bk = np.asarray(inputs["bk"], dtype=np.float32)
    bv = np.asarray(inputs["bv"], dtype=np.float32)

    # x^T per batch, bf16, contiguous: [D, S]
    xT = [np.ascontiguousarray(x[b].T.astype(bf16)) for b in range(B)]

    def w_cols(W, cols):
        # [D, 256] -> SBUF layout [128, 8, 256] flattened to [128, 2048]
        w = W[:, cols].astype(bf16)
        return np.ascontiguousarray(
            w.reshape(NM, 128, OC).transpose(1, 0, 2).reshape(128, NM * OC))

    def w_rows(W, rows):
        # [256, D] -> SBUF layout [128, 2, 1024] flattened to [128, 2048]
        w = W[rows, :].astype(bf16)
        return np.ascontiguousarray(
            w.reshape(2, 128, D).transpose(1, 0, 2).reshape(128, 2 * D))

    # per-head-group tensors are shared by the two batch cores (c and c+4)
    gmaps = []
    for g in range(4):
        cols = slice(g * OC, (g + 1) * OC)
        gmaps.append({
            "wq": w_cols(Wq, cols),
            "wk": w_cols(Wk, cols),
            "wv": w_cols(Wv, cols),
            "wo": w_rows(Wo, cols),
            "bq": np.ascontiguousarray(bq[cols]),
            "bk": np.ascontiguousarray(bk[cols]),
            "bv": np.ascontiguousarray(bv[cols]),
        })
    return [{"xt": xT[c // 4], **gmaps[c % 4]} for c in range(8)]


def combine(results, inputs):
    bo = np.asarray(inputs["bo"], dtype=np.float32)
    out = np.zeros((B, S, D), dtype=np.float32)
    for c in range(8):
        out[c // 4] += results[c]["out"].astype(np.float32)
    out += bo[None, None, :]
    return out


def kernel(**inputs) -> np.ndarray:
    nc = _get_nc()
    in_maps = make_in_maps(inputs)
    res = run_bass_kernel_spmd(nc, in_maps, core_ids=list(range(8)))
    return combine(res.results, inputs)


if __name__ == "__main__":
    import jax
    print(jax.devices())
